# revision 4
# baseline (speedup 1.0000x reference)
"""AdaPT int8-quantized Linear on 8 TRN2 NeuronCores.

out = round_int8(x * 127/amax(x)) @ round_int8(w * 127/amax(w)).T * dequant + bias

Exactness: int8 values (|v| <= 127) are exact in bf16; their products
(<= 16129) and the accumulated partial sums (~1e5 << 2^24) are exact in
fp32 PSUM, so a bf16 TensorE matmul reproduces the int8 x int8 -> int32
matmul bit-exactly at full bf16 throughput. round() is implemented as
(v*scale + 1.5*2^23) - 1.5*2^23 in f32 (round-half-even, matching jnp).

Sharding: x row-parallel (1024 rows/core). Each core quantizes a distinct
512-row slice of w; the quantized bf16 w is AllGathered in 4 pipelined
chunks (128 rows/rank each) so the first matmul panel is ready ~20us after
quantization instead of waiting for a monolithic 193us AllGather. The
chunk interleaving permutes w rows; a strided output-DMA access pattern
un-permutes on the way out (bias is pre-permuted on the host to match).
amax is computed on-device (per-core abs-max over distinct slices +
AllReduce-max collective of 2 scalars).
"""

import numpy as np

import concourse.bass as bass
import concourse.bacc as bacc
import concourse.bass_isa as bass_isa
import concourse.mybir as mybir
import concourse.tile as tile
from concourse.bass_utils import run_bass_kernel_spmd

N, K, M = 8192, 4096, 4096
N_CORES = 8
NS = N // N_CORES   # 1024 x rows per core
WS = M // N_CORES   # 512 w rows per core (quantize shard)
P = 128
KB = K // P         # 32 k-blocks
NB = NS // P        # 8 n-blocks per core
MP = 512            # m-panel width
NMP = M // MP       # 8 m-panels
XT = NS // P        # 8 x f32 tiles
WT = WS // P        # 4 w f32 tiles (= AllGather chunks)

MAGIC = 12582912.0  # 1.5 * 2**23
F32 = mybir.dt.float32
BF16 = mybir.dt.bfloat16

_cached_nc = None


def _bias_perm():
    # device writes panel p (=2j+q), free index f=rp*128+i to output column
    # 2048q + 512rp + 128j + i; bias_bc[:, p*512+f] must hold bias[that col]
    p = np.arange(NMP)[:, None, None]
    rp = np.arange(4)[None, :, None]
    i = np.arange(P)[None, None, :]
    col = 2048 * (p % 2) + 512 * rp + 128 * (p // 2) + i
    return col.reshape(-1)


def _body(nc, tc, xs, wa, bias_in, out):
    RG = [list(range(N_CORES))]
    xs_t = xs.rearrange("(t p) k -> t p k", p=P)
    wa_t = wa.rearrange("(t p) k -> t p k", p=P)

    with (
        tc.tile_pool(name="const", bufs=1) as const,
        tc.tile_pool(name="dram", bufs=1, space="DRAM") as dram,
        tc.tile_pool(name="ld", bufs=3) as ld,
        tc.tile_pool(name="qb", bufs=2) as qbp,
        tc.tile_pool(name="xt", bufs=NB) as xtp,
        tc.tile_pool(name="wt", bufs=3) as wtp,
        tc.tile_pool(name="ps", bufs=4, space="PSUM") as psp,
        tc.tile_pool(name="ob", bufs=3) as obp,
    ):
        wq_c = [dram.tile([P, K], BF16, name=f"wq_c{j}") for j in range(WT)]
        wg = [dram.tile([4 * P * 2, K], BF16, addr_space="Shared", name=f"wg{j}")
              for j in range(WT)]
        xq = dram.tile([NS, K], BF16)
        cc_in = dram.tile([1, 16], F32)
        cc_out = dram.tile([1, 16], F32, addr_space="Shared")

        # bias (host-permuted) broadcast across all 128 partitions
        bias_bc = const.tile([P, M], F32)
        bias_b_ap = bass.AP(
            tensor=bias_in.tensor,
            offset=bias_in.offset,
            ap=[[0, P]] + list(bias_in.ap),
        )
        nc.gpsimd.dma_start(out=bias_bc[:], in_=bias_b_ap)

        scl = const.tile([P, 4], F32)  # 0:scale_x 1:scale_w 2:dequant 3:tmp

        # ---- Phase A: local abs-max over this core's distinct slices ----
        partx = const.tile([P, XT], F32)
        partw = const.tile([P, WT], F32)
        for t in range(WT):
            tl = ld.tile([P, K], F32, tag="ldf32", name=f"ldw{t}")
            nc.scalar.dma_start(tl[:], wa_t[t])
            nc.vector.tensor_reduce(
                out=partw[:, t : t + 1], in_=tl[:], op=mybir.AluOpType.max,
                axis=mybir.AxisListType.X, apply_absolute_value=True,
            )
        for t in range(XT):
            tl = ld.tile([P, K], F32, tag="ldf32", name=f"ldx{t}")
            nc.sync.dma_start(tl[:], xs_t[t])
            nc.vector.tensor_reduce(
                out=partx[:, t : t + 1], in_=tl[:], op=mybir.AluOpType.max,
                axis=mybir.AxisListType.X, apply_absolute_value=True,
            )
        pxw = const.tile([P, 2], F32)
        nc.vector.tensor_reduce(out=pxw[:, 0:1], in_=partx[:], op=mybir.AluOpType.max,
                                axis=mybir.AxisListType.X)
        nc.vector.tensor_reduce(out=pxw[:, 1:2], in_=partw[:], op=mybir.AluOpType.max,
                                axis=mybir.AxisListType.X)
        rxw = const.tile([P, 2], F32)
        nc.gpsimd.partition_all_reduce(rxw[:], pxw[:], channels=P,
                                       reduce_op=bass_isa.ReduceOp.max)

        # ---- AllReduce(max) of [amax_x, amax_w] across the 8 cores ----
        pack = const.tile([1, 16], F32)
        nc.vector.memset(pack[:], 0.0)
        nc.vector.tensor_copy(pack[:1, 0:2], rxw[:1, 0:2])
        nc.gpsimd.dma_start(cc_in[:], pack[:])
        nc.gpsimd.collective_compute(
            "AllReduce", mybir.AluOpType.max,
            ins=[cc_in.opt()], outs=[cc_out.opt()], replica_groups=RG,
        )
        got = const.tile([1, 16], F32)
        nc.gpsimd.dma_start(got[:], cc_out[:])
        gb = const.tile([P, 16], F32)
        nc.gpsimd.partition_broadcast(gb[:], got[:])

        # scale_x = 127/amax_x ; scale_w = 127/amax_w (reciprocal+mult)
        inv = const.tile([P, 2], F32)
        nc.vector.reciprocal(inv[:], gb[:, 0:2])
        nc.vector.tensor_scalar(out=scl[:, 0:2], in0=inv[:], scalar1=127.0,
                                scalar2=None, op0=mybir.AluOpType.mult)
        # dequant = amax_x * amax_w * (1/16129)
        nc.vector.tensor_tensor(out=scl[:, 3:4], in0=gb[:, 0:1], in1=gb[:, 1:2],
                                op=mybir.AluOpType.mult)
        nc.vector.tensor_scalar(out=scl[:, 2:3], in0=scl[:, 3:4],
                                scalar1=float(np.float32(1.0) / np.float32(16129.0)),
                                scalar2=None, op0=mybir.AluOpType.mult)

        xq_t = xq.rearrange("(t p) k -> t p k", p=P)
        xT = [None] * XT

        def quant_w_chunk(j):
            tl = ld.tile([P, K], F32, tag="ldf32", name=f"ldw2{j}")
            nc.sync.dma_start(tl[:], wa_t[j])
            nc.vector.tensor_scalar(out=tl[:], in0=tl[:], scalar1=scl[:, 1:2],
                                    scalar2=MAGIC, op0=mybir.AluOpType.mult,
                                    op1=mybir.AluOpType.add)
            q = qbp.tile([P, K], BF16, tag="qb", name=f"wqb{j}")
            nc.vector.tensor_scalar(out=q[:], in0=tl[:], scalar1=MAGIC,
                                    scalar2=None, op0=mybir.AluOpType.subtract)
            nc.gpsimd.dma_start(wq_c[j][:], q[:])
            nc.gpsimd.collective_compute(
                "AllGather", mybir.AluOpType.bypass,
                ins=[wq_c[j].opt()], outs=[wg[j].opt()], replica_groups=RG,
            )

        def quant_x_tile(t):
            tl = ld.tile([P, K], F32, tag="ldf32", name=f"ldx2{t}")
            nc.scalar.dma_start(tl[:], xs_t[t])
            nc.vector.tensor_scalar(out=tl[:], in0=tl[:], scalar1=scl[:, 0:1],
                                    scalar2=MAGIC, op0=mybir.AluOpType.mult,
                                    op1=mybir.AluOpType.add)
            q = qbp.tile([P, K], BF16, tag="qb", name=f"xqb{t}")
            nc.vector.tensor_scalar(out=q[:], in0=tl[:], scalar1=MAGIC,
                                    scalar2=None, op0=mybir.AluOpType.subtract)
            nc.gpsimd.dma_start(xq_t[t], q[:])
            xT[t] = xtp.tile([P, KB, P], BF16, tag="xT", name=f"xT{t}")
            nc.sync.dma_start_transpose(xT[t][:], xq[t * P : (t + 1) * P, :])

        # DVE/queue ordering: w chunk 0 first (gates AG0 -> first panel),
        # then x tile 0 (gates xT0), then the rest interleaved.
        quant_w_chunk(0)
        quant_x_tile(0)
        quant_w_chunk(1)
        quant_x_tile(1)
        quant_w_chunk(2)
        quant_x_tile(2)
        quant_w_chunk(3)
        for t in range(3, XT):
            quant_x_tile(t)

        # ---- main loop: transposed wT loads + matmuls + fused dequant/bias ----
        for p in range(NMP):
            j, q = p // 2, p % 2
            wth = []
            for h in range(2):
                w = wtp.tile([P, KB // 2, MP], BF16, tag="wT", name=f"wT{p}_{h}")
                nc.sync.dma_start_transpose(
                    w[:], wg[j][q * 512 : (q + 1) * 512, h * 2048 : (h + 1) * 2048]
                )
                wth.append(w)
            for nb in range(NB):
                ps = psp.tile([P, MP], F32, tag="ps", name=f"ps{p}_{nb}")
                for ks in range(KB):
                    nc.tensor.matmul(
                        ps[:], xT[nb][:, ks, :], wth[ks // 16][:, ks % 16, :],
                        start=(ks == 0), stop=(ks == KB - 1),
                    )
                ob = obp.tile([P, MP], F32, tag="ob", name=f"ob{p}_{nb}")
                nc.vector.scalar_tensor_tensor(
                    out=ob[:], in0=ps[:], scalar=scl[:, 2:3],
                    in1=bias_bc[:, p * MP : (p + 1) * MP],
                    op0=mybir.AluOpType.mult, op1=mybir.AluOpType.add,
                )
                out_ap = bass.AP(
                    tensor=out.tensor,
                    offset=out.offset + nb * P * M + q * 2048 + j * P,
                    ap=[[M, P], [512, 4], [1, P]],
                )
                nc.gpsimd.dma_start(out_ap, ob[:])


def _build():
    global _cached_nc
    if _cached_nc is not None:
        return _cached_nc
    nc = bacc.Bacc("TRN2", target_bir_lowering=False, debug=False,
                   num_devices=N_CORES)
    xs = nc.dram_tensor("xs", [NS, K], F32, kind="ExternalInput")
    wa = nc.dram_tensor("wa", [WS, K], F32, kind="ExternalInput")
    bias = nc.dram_tensor("bias", [M], F32, kind="ExternalInput")
    out = nc.dram_tensor("out", [NS, M], F32, kind="ExternalOutput")
    with tile.TileContext(nc) as tc:
        _body(nc, tc, xs.ap(), wa.ap(), bias.ap(), out.ap())
    nc.compile()
    _cached_nc = nc
    return nc


def kernel(x, weight, bias, _trace=False, _trace_kwargs=None):
    x = np.ascontiguousarray(np.asarray(x, dtype=np.float32))
    weight = np.ascontiguousarray(np.asarray(weight, dtype=np.float32))
    bias = np.ascontiguousarray(np.asarray(bias, dtype=np.float32))
    assert x.shape == (N, K) and weight.shape == (M, K) and bias.shape == (M,)

    nc = _build()
    bias_p = np.ascontiguousarray(bias[_bias_perm()])
    in_maps = [
        {
            "xs": x[c * NS : (c + 1) * NS],
            "wa": weight[c * WS : (c + 1) * WS],
            "bias": bias_p,
        }
        for c in range(N_CORES)
    ]
    res = run_bass_kernel_spmd(
        nc, in_maps, core_ids=list(range(N_CORES)),
        trace=_trace, **(_trace_kwargs or {}),
    )
    out = np.concatenate([res.results[c]["out"] for c in range(N_CORES)], axis=0)
    if _trace:
        return out, res
    return out


# revision 5
# speedup vs baseline: 1.1276x; 1.1276x over previous
"""AdaPT int8-quantized Linear on 8 TRN2 NeuronCores.

out = round_int8(x * 127/amax(x)) @ round_int8(w * 127/amax(w)).T * dequant + bias

Exactness: int8 values (|v| <= 127) are exact in bf16; their products
(<= 16129) and the accumulated partial sums (~1e5 << 2^24) are exact in
fp32 PSUM, so a bf16 TensorE matmul reproduces the int8 x int8 -> int32
matmul bit-exactly at full bf16 throughput. round() is implemented as
(v*scale + 1.5*2^23) - 1.5*2^23 in f32 (round-half-even, matching jnp).

Layout strategy: the TensorEngine contracts along the partition axis, so
both operands need k-major layout. Rather than transposing on device
(xbar transposes serialize against collectives in the Tile scheduler),
kernel() passes x.T and w.T slices (numpy prep). The device then only
does contiguous loads, elementwise quantize, matmuls, and stores.

Sharding: x column...rows row-parallel (1024 rows/core; xs = x.T slice).
Each core quantizes a distinct 512-column slice of w.T; the quantized
bf16 w is AllGathered in 4 k-chunks pipelined with the matmul panels.
amax is computed on-device (per-core abs-max over distinct slices +
AllReduce-max collective of 2 scalars).
"""

import numpy as np

import concourse.bass as bass
import concourse.bacc as bacc
import concourse.bass_isa as bass_isa
import concourse.mybir as mybir
import concourse.tile as tile
from concourse.bass_utils import run_bass_kernel_spmd

N, K, M = 8192, 4096, 4096
N_CORES = 8
NS = N // N_CORES   # 1024 x rows per core
WS = M // N_CORES   # 512 w rows per core (quantize shard)
P = 128
KB = K // P         # 32 k-blocks
NB = NS // P        # 8 n-blocks per core
MP = 512            # m-panel width
NMP = M // MP       # 8 m-panels
NCH = 4             # AllGather k-chunks
KCH = K // NCH      # 1024 k rows per chunk (8 k-blocks)

MAGIC = 12582912.0  # 1.5 * 2**23
F32 = mybir.dt.float32
BF16 = mybir.dt.bfloat16

_cached_nc = None


def _body(nc, tc, xs, wa, bias_in, out):
    RG = [list(range(N_CORES))]
    # xs: [K, NS] f32 (x.T slice)  -> load tiles [128, 4, NS], k-major
    # wa: [K, WS] f32 (w.T slice)  -> load tiles [128, 8, MP], k-major
    xs_t = xs.rearrange("(t a p) n -> t p a n", a=4, p=P)   # [8, 128, 4, 1024]
    wa_t = wa.rearrange("(h j p) m -> h p j m", j=8, p=P)   # [4, 128, 8, 512]

    with (
        tc.tile_pool(name="const", bufs=1) as const,
        tc.tile_pool(name="dram", bufs=1, space="DRAM") as dram,
        tc.tile_pool(name="ld", bufs=3) as ld,
        tc.tile_pool(name="wqs", bufs=2) as wqsp,
        tc.tile_pool(name="xt", bufs=1) as xtp,
        tc.tile_pool(name="wt", bufs=5) as wtp,
        tc.tile_pool(name="ps", bufs=4, space="PSUM") as psp,
        tc.tile_pool(name="ob", bufs=3) as obp,
    ):
        wq_c = [dram.tile([KCH, WS], BF16, name=f"wq_c{h}") for h in range(NCH)]
        # AllGather out: rank r's [KCH, WS] block at [r*KCH:(r+1)*KCH, :]
        wg = [dram.tile([N_CORES * KCH, WS], BF16, addr_space="Shared",
                        name=f"wg{h}") for h in range(NCH)]
        cc_in = dram.tile([1, 16], F32)
        cc_out = dram.tile([1, 16], F32, addr_space="Shared")

        bias_bc = const.tile([P, M], F32)
        bias_b_ap = bass.AP(
            tensor=bias_in.tensor,
            offset=bias_in.offset,
            ap=[[0, P]] + list(bias_in.ap),
        )
        nc.gpsimd.dma_start(out=bias_bc[:], in_=bias_b_ap)

        scl = const.tile([P, 4], F32)  # 0:scale_x 1:scale_w 2:dequant 3:tmp

        # ---- Phase A: local abs-max over this core's distinct slices ----
        partx = const.tile([P, 8], F32)
        partw = const.tile([P, NCH], F32)
        for h in range(NCH):
            tl = ld.tile([P, 8, MP], F32, tag="ldf32", name=f"ldw{h}")
            nc.scalar.dma_start(tl[:], wa_t[h])
            nc.vector.tensor_reduce(
                out=partw[:, h : h + 1], in_=tl[:], op=mybir.AluOpType.max,
                axis=mybir.AxisListType.XY, apply_absolute_value=True,
            )
        for t in range(8):
            tl = ld.tile([P, 4, NS], F32, tag="ldf32", name=f"ldx{t}")
            nc.sync.dma_start(tl[:], xs_t[t])
            nc.vector.tensor_reduce(
                out=partx[:, t : t + 1], in_=tl[:], op=mybir.AluOpType.max,
                axis=mybir.AxisListType.XY, apply_absolute_value=True,
            )
        pxw = const.tile([P, 2], F32)
        nc.vector.tensor_reduce(out=pxw[:, 0:1], in_=partx[:], op=mybir.AluOpType.max,
                                axis=mybir.AxisListType.X)
        nc.vector.tensor_reduce(out=pxw[:, 1:2], in_=partw[:], op=mybir.AluOpType.max,
                                axis=mybir.AxisListType.X)
        rxw = const.tile([P, 2], F32)
        nc.gpsimd.partition_all_reduce(rxw[:], pxw[:], channels=P,
                                       reduce_op=bass_isa.ReduceOp.max)

        # ---- AllReduce(max) of [amax_x, amax_w] across the 8 cores ----
        pack = const.tile([1, 16], F32)
        nc.vector.memset(pack[:], 0.0)
        nc.vector.tensor_copy(pack[:1, 0:2], rxw[:1, 0:2])
        nc.gpsimd.dma_start(cc_in[:], pack[:])
        nc.gpsimd.collective_compute(
            "AllReduce", mybir.AluOpType.max,
            ins=[cc_in.opt()], outs=[cc_out.opt()], replica_groups=RG,
        )
        got = const.tile([1, 16], F32)
        nc.gpsimd.dma_start(got[:], cc_out[:])
        gb = const.tile([P, 16], F32)
        nc.gpsimd.partition_broadcast(gb[:], got[:])

        inv = const.tile([P, 2], F32)
        nc.vector.reciprocal(inv[:], gb[:, 0:2])
        nc.vector.tensor_scalar(out=scl[:, 0:2], in0=inv[:], scalar1=127.0,
                                scalar2=None, op0=mybir.AluOpType.mult)
        nc.vector.tensor_tensor(out=scl[:, 3:4], in0=gb[:, 0:1], in1=gb[:, 1:2],
                                op=mybir.AluOpType.mult)
        nc.vector.tensor_scalar(out=scl[:, 2:3], in0=scl[:, 3:4],
                                scalar1=float(np.float32(1.0) / np.float32(16129.0)),
                                scalar2=None, op0=mybir.AluOpType.mult)

        xT = xtp.tile([P, KB, NS], BF16)  # resident quantized x.T (8.4 MB)
        wq_cv = [wq_c[h].rearrange("(j p) m -> p j m", p=P) for h in range(NCH)]

        def quant_w_chunk(h):
            tl = ld.tile([P, 8, MP], F32, tag="ldf32", name=f"ldw2{h}")
            nc.scalar.dma_start(tl[:], wa_t[h])
            nc.vector.tensor_scalar(out=tl[:], in0=tl[:], scalar1=scl[:, 1:2],
                                    scalar2=MAGIC, op0=mybir.AluOpType.mult,
                                    op1=mybir.AluOpType.add)
            q = wqsp.tile([P, 8, MP], BF16, tag="wqs", name=f"wqb{h}")
            nc.vector.tensor_scalar(out=q[:], in0=tl[:], scalar1=MAGIC,
                                    scalar2=None, op0=mybir.AluOpType.subtract)
            nc.gpsimd.dma_start(wq_cv[h], q[:])
            nc.gpsimd.collective_compute(
                "AllGather", mybir.AluOpType.bypass,
                ins=[wq_c[h].opt()], outs=[wg[h].opt()], replica_groups=RG,
            )

        def quant_x_tile(t):
            tl = ld.tile([P, 4, NS], F32, tag="ldf32", name=f"ldx2{t}")
            nc.sync.dma_start(tl[:], xs_t[t])
            nc.vector.tensor_scalar(out=tl[:], in0=tl[:], scalar1=scl[:, 0:1],
                                    scalar2=MAGIC, op0=mybir.AluOpType.mult,
                                    op1=mybir.AluOpType.add)
            nc.vector.tensor_scalar(out=xT[:, 4 * t : 4 * t + 4, :], in0=tl[:],
                                    scalar1=MAGIC, scalar2=None,
                                    op0=mybir.AluOpType.subtract)

        quant_w_chunk(0)
        quant_x_tile(0)
        quant_x_tile(1)
        quant_w_chunk(1)
        quant_x_tile(2)
        quant_x_tile(3)
        quant_w_chunk(2)
        quant_x_tile(4)
        quant_x_tile(5)
        quant_w_chunk(3)
        quant_x_tile(6)
        quant_x_tile(7)

        # ---- main loop: plain k-major moving loads + matmuls + epilogue ----
        for p in range(NMP):
            wth = []
            for h in range(NCH):
                w = wtp.tile([P, 8, MP], BF16, tag="wT", name=f"wT{p}_{h}")
                # rank-p block of AllGather chunk h, k-major [KCH, MP]
                src = bass.AP(
                    tensor=wg[h].tensor,
                    offset=wg[h].offset + p * KCH * MP,
                    ap=[[MP, P], [P * MP, 8], [1, MP]],
                )
                nc.scalar.dma_start(w[:], src)
                wth.append(w)
            for nb in range(NB):
                ps = psp.tile([P, MP], F32, tag="ps", name=f"ps{p}_{nb}")
                for ks in range(KB):
                    nc.tensor.matmul(
                        ps[:], xT[:, ks, nb * P : (nb + 1) * P],
                        wth[ks // 8][:, ks % 8, :],
                        start=(ks == 0), stop=(ks == KB - 1),
                    )
                ob = obp.tile([P, MP], F32, tag="ob", name=f"ob{p}_{nb}")
                nc.vector.scalar_tensor_tensor(
                    out=ob[:], in0=ps[:], scalar=scl[:, 2:3],
                    in1=bias_bc[:, p * MP : (p + 1) * MP],
                    op0=mybir.AluOpType.mult, op1=mybir.AluOpType.add,
                )
                nc.gpsimd.dma_start(
                    out[nb * P : (nb + 1) * P, p * MP : (p + 1) * MP], ob[:]
                )


def _build():
    global _cached_nc
    if _cached_nc is not None:
        return _cached_nc
    nc = bacc.Bacc("TRN2", target_bir_lowering=False, debug=False,
                   num_devices=N_CORES)
    xs = nc.dram_tensor("xs", [K, NS], F32, kind="ExternalInput")
    wa = nc.dram_tensor("wa", [K, WS], F32, kind="ExternalInput")
    bias = nc.dram_tensor("bias", [M], F32, kind="ExternalInput")
    out = nc.dram_tensor("out", [NS, M], F32, kind="ExternalOutput")
    with tile.TileContext(nc) as tc:
        _body(nc, tc, xs.ap(), wa.ap(), bias.ap(), out.ap())
    nc.compile()
    _cached_nc = nc
    return nc


def kernel(x, weight, bias, _trace=False, _trace_kwargs=None):
    x = np.asarray(x, dtype=np.float32)
    weight = np.asarray(weight, dtype=np.float32)
    bias = np.ascontiguousarray(np.asarray(bias, dtype=np.float32))
    assert x.shape == (N, K) and weight.shape == (M, K) and bias.shape == (M,)

    nc = _build()
    xt = x.T  # [K, N] view
    wt = weight.T  # [K, M] view
    in_maps = [
        {
            "xs": np.ascontiguousarray(xt[:, c * NS : (c + 1) * NS]),
            "wa": np.ascontiguousarray(wt[:, c * WS : (c + 1) * WS]),
            "bias": bias,
        }
        for c in range(N_CORES)
    ]
    res = run_bass_kernel_spmd(
        nc, in_maps, core_ids=list(range(N_CORES)),
        trace=_trace, **(_trace_kwargs or {}),
    )
    out = np.concatenate([res.results[c]["out"] for c in range(N_CORES)], axis=0)
    if _trace:
        return out, res
    return out


# revision 7
# speedup vs baseline: 1.3191x; 1.1698x over previous
"""AdaPT int8-quantized Linear on 8 TRN2 NeuronCores.

out = round_int8(x * 127/amax(x)) @ round_int8(w * 127/amax(w)).T * dequant + bias

Exactness: int8 values (|v| <= 127) are exact in bf16; their products
(<= 16129) and the accumulated partial sums (~1e5 << 2^24) are exact in
fp32 PSUM, so a bf16 TensorE matmul reproduces the int8 x int8 -> int32
matmul bit-exactly at full bf16 throughput. round() is implemented as
(v*scale + 1.5*2^23) - 1.5*2^23 in f32 (round-half-even, matching jnp).

Layout strategy: the TensorEngine contracts along the partition axis, so
both operands need k-major layout. kernel() passes x.T / w.T slices
(numpy prep); the device does only contiguous/strided loads, elementwise
quantize (ScalarE scale+magic, VectorE subtract+bf16-cast), matmuls and
stores. No device transposes, no large collectives: each core re-reads
all of w.T and quantizes it panel-by-panel, pipelined under the matmuls.
amax is computed on-device: per-core abs-max over disjoint slices of x
and w (VectorE for x, GpSimd full-reduce for w), then one AllReduce-max
collective of 2 scalars.

x row-parallel: core c computes out rows [c*1024, (c+1)*1024).
"""

import numpy as np

import concourse.bass as bass
import concourse.bacc as bacc
import concourse.bass_isa as bass_isa
import concourse.mybir as mybir
import concourse.tile as tile
from concourse.bass_utils import run_bass_kernel_spmd

N, K, M = 8192, 4096, 4096
N_CORES = 8
NS = N // N_CORES   # 1024 x rows per core
WS = M // N_CORES   # 512 w rows per core (amax shard)
P = 128
KB = K // P         # 32 k-blocks
NB = NS // P        # 8 n-blocks per core
MP = 512            # m-panel width
NMP = M // MP       # 8 m-panels

MAGIC = 12582912.0  # 1.5 * 2**23
F32 = mybir.dt.float32
BF16 = mybir.dt.bfloat16

_cached_nc = None


def _body(nc, tc, xs, wa, wf, bias_in, out):
    RG = [list(range(N_CORES))]
    # xs: [K, NS] f32 (x.T slice)   -> tiles [128, 4, NS], k on partitions
    # wa: [K, WS] f32 (w.T slice)   -> tiles [128, 8, MP] (amax only)
    # wf: [K, M]  f32 (full w.T)    -> per-panel tiles [128, 8, MP]
    xs_t = xs.rearrange("(t a p) n -> t p a n", a=4, p=P)   # [8, 128, 4, 1024]
    wa_t = wa.rearrange("(h j p) m -> h p j m", j=8, p=P)   # [4, 128, 8, 512]

    with (
        tc.tile_pool(name="const", bufs=1) as const,
        tc.tile_pool(name="dram", bufs=1, space="DRAM") as dram,
        tc.tile_pool(name="ld", bufs=3) as ld,
        tc.tile_pool(name="xt", bufs=1) as xtp,
        tc.tile_pool(name="wt", bufs=6) as wtp,
        tc.tile_pool(name="ps", bufs=4, space="PSUM") as psp,
        tc.tile_pool(name="ob", bufs=3) as obp,
    ):
        cc_in = dram.tile([1, 16], F32)
        cc_out = dram.tile([1, 16], F32, addr_space="Shared")

        bias_bc = const.tile([P, M], F32)
        bias_b_ap = bass.AP(
            tensor=bias_in.tensor,
            offset=bias_in.offset,
            ap=[[0, P]] + list(bias_in.ap),
        )
        nc.gpsimd.dma_start(out=bias_bc[:], in_=bias_b_ap)

        scl = const.tile([P, 4], F32)   # 0:scale_x 1:scale_w 2:dequant 3:tmp
        magic = const.tile([P, 1], F32)
        nc.vector.memset(magic[:], MAGIC)

        # ---- Phase A: local abs-max over this core's distinct slices ----
        partx = const.tile([P, 8], F32)
        partw = const.tile([P, 4], F32)
        for h in range(4):
            tl = ld.tile([P, 8, MP], F32, tag="ldf32", name=f"ldw{h}")
            nc.scalar.dma_start(tl[:], wa_t[h])
            nc.vector.tensor_reduce(
                out=partw[:, h : h + 1], in_=tl[:], op=mybir.AluOpType.max,
                axis=mybir.AxisListType.XY, apply_absolute_value=True,
            )
        for t in range(8):
            tl = ld.tile([P, 4, NS], F32, tag="ldf32", name=f"ldx{t}")
            nc.sync.dma_start(tl[:], xs_t[t])
            nc.vector.tensor_reduce(
                out=partx[:, t : t + 1], in_=tl[:], op=mybir.AluOpType.max,
                axis=mybir.AxisListType.XY, apply_absolute_value=True,
            )
        pxw = const.tile([P, 2], F32)
        nc.vector.tensor_reduce(out=pxw[:, 0:1], in_=partx[:], op=mybir.AluOpType.max,
                                axis=mybir.AxisListType.X)
        nc.vector.tensor_reduce(out=pxw[:, 1:2], in_=partw[:], op=mybir.AluOpType.max,
                                axis=mybir.AxisListType.X)
        rxw = const.tile([P, 2], F32)
        nc.gpsimd.partition_all_reduce(rxw[:], pxw[:], channels=P,
                                       reduce_op=bass_isa.ReduceOp.max)

        # ---- AllReduce(max) of [amax_x, amax_w] across the 8 cores ----
        pack = const.tile([1, 16], F32)
        nc.vector.memset(pack[:], 0.0)
        nc.vector.tensor_copy(pack[:1, 0:2], rxw[:1, 0:2])
        nc.gpsimd.dma_start(cc_in[:], pack[:])
        nc.gpsimd.collective_compute(
            "AllReduce", mybir.AluOpType.max,
            ins=[cc_in.opt()], outs=[cc_out.opt()], replica_groups=RG,
        )
        got = const.tile([1, 16], F32)
        nc.gpsimd.dma_start(got[:], cc_out[:])
        gb = const.tile([P, 16], F32)
        nc.gpsimd.partition_broadcast(gb[:], got[:])

        inv = const.tile([P, 2], F32)
        nc.vector.reciprocal(inv[:], gb[:, 0:2])
        nc.vector.tensor_scalar(out=scl[:, 0:2], in0=inv[:], scalar1=127.0,
                                scalar2=None, op0=mybir.AluOpType.mult)
        nc.vector.tensor_tensor(out=scl[:, 3:4], in0=gb[:, 0:1], in1=gb[:, 1:2],
                                op=mybir.AluOpType.mult)
        nc.vector.tensor_scalar(out=scl[:, 2:3], in0=scl[:, 3:4],
                                scalar1=float(np.float32(1.0) / np.float32(16129.0)),
                                scalar2=None, op0=mybir.AluOpType.mult)

        xT = xtp.tile([P, KB, NS], BF16)  # resident quantized x.T (8.4 MB)

        def quant_x_tile(t):
            tl = ld.tile([P, 4, NS], F32, tag="ldf32", name=f"ldx2{t}")
            nc.sync.dma_start(tl[:], xs_t[t])
            nc.vector.tensor_scalar(out=tl[:], in0=tl[:], scalar1=scl[:, 0:1],
                                    scalar2=MAGIC, op0=mybir.AluOpType.mult,
                                    op1=mybir.AluOpType.add)
            nc.vector.tensor_scalar(out=xT[:, 4 * t : 4 * t + 4, :], in0=tl[:],
                                    scalar1=MAGIC, scalar2=None,
                                    op0=mybir.AluOpType.subtract)

        for t in range(8):
            quant_x_tile(t)

        # ---- main loop: per-panel w load+quantize, matmuls, epilogue ----
        for p in range(NMP):
            wth = []
            for h in range(4):
                tl = ld.tile([P, 8, MP], F32, tag="ldf32", name=f"ldwp{p}_{h}")
                src = bass.AP(
                    tensor=wf.tensor,
                    offset=wf.offset + h * (K // 4) * M + p * MP,
                    ap=[[M, P], [P * M, 8], [1, MP]],
                )
                nc.scalar.dma_start(tl[:], src)
                nc.vector.tensor_scalar(out=tl[:], in0=tl[:], scalar1=scl[:, 1:2],
                                        scalar2=MAGIC, op0=mybir.AluOpType.mult,
                                        op1=mybir.AluOpType.add)
                w = wtp.tile([P, 8, MP], BF16, tag="wT", name=f"wT{p}_{h}")
                nc.vector.tensor_scalar(out=w[:], in0=tl[:], scalar1=MAGIC,
                                        scalar2=None, op0=mybir.AluOpType.subtract)
                wth.append(w)
            for nb in range(NB):
                ps = psp.tile([P, MP], F32, tag="ps", name=f"ps{p}_{nb}")
                for i in range(KB):
                    ks = (4 * nb + i) % KB
                    nc.tensor.matmul(
                        ps[:], xT[:, ks, nb * P : (nb + 1) * P],
                        wth[ks // 8][:, ks % 8, :],
                        start=(i == 0), stop=(i == KB - 1),
                    )
                ob = obp.tile([P, MP], F32, tag="ob", name=f"ob{p}_{nb}")
                nc.vector.scalar_tensor_tensor(
                    out=ob[:], in0=ps[:], scalar=scl[:, 2:3],
                    in1=bias_bc[:, p * MP : (p + 1) * MP],
                    op0=mybir.AluOpType.mult, op1=mybir.AluOpType.add,
                )
                nc.gpsimd.dma_start(
                    out[nb * P : (nb + 1) * P, p * MP : (p + 1) * MP], ob[:]
                )


def _build():
    global _cached_nc
    if _cached_nc is not None:
        return _cached_nc
    nc = bacc.Bacc("TRN2", target_bir_lowering=False, debug=False,
                   num_devices=N_CORES)
    xs = nc.dram_tensor("xs", [K, NS], F32, kind="ExternalInput")
    wa = nc.dram_tensor("wa", [K, WS], F32, kind="ExternalInput")
    wf = nc.dram_tensor("wf", [K, M], F32, kind="ExternalInput")
    bias = nc.dram_tensor("bias", [M], F32, kind="ExternalInput")
    out = nc.dram_tensor("out", [NS, M], F32, kind="ExternalOutput")
    with tile.TileContext(nc) as tc:
        _body(nc, tc, xs.ap(), wa.ap(), wf.ap(), bias.ap(), out.ap())
    nc.compile()
    _cached_nc = nc
    return nc


def kernel(x, weight, bias, _trace=False, _trace_kwargs=None):
    x = np.asarray(x, dtype=np.float32)
    weight = np.asarray(weight, dtype=np.float32)
    bias = np.ascontiguousarray(np.asarray(bias, dtype=np.float32))
    assert x.shape == (N, K) and weight.shape == (M, K) and bias.shape == (M,)

    nc = _build()
    xt = x.T          # [K, N] view
    wt = np.ascontiguousarray(weight.T)  # [K, M]
    in_maps = [
        {
            "xs": np.ascontiguousarray(xt[:, c * NS : (c + 1) * NS]),
            "wa": np.ascontiguousarray(wt[:, c * WS : (c + 1) * WS]),
            "wf": wt,
            "bias": bias,
        }
        for c in range(N_CORES)
    ]
    res = run_bass_kernel_spmd(
        nc, in_maps, core_ids=list(range(N_CORES)),
        trace=_trace, **(_trace_kwargs or {}),
    )
    out = np.concatenate([res.results[c]["out"] for c in range(N_CORES)], axis=0)
    if _trace:
        return out, res
    return out


# revision 8
# speedup vs baseline: 1.3330x; 1.0105x over previous
"""AdaPT int8-quantized Linear on 8 TRN2 NeuronCores.

out = round_int8(x * 127/amax(x)) @ round_int8(w * 127/amax(w)).T * dequant + bias

Exactness: int8 values (|v| <= 127) are exact in bf16; their products
(<= 16129) and the accumulated partial sums (~1e5 << 2^24) are exact in
fp32 PSUM, so a bf16 TensorE matmul reproduces the int8 x int8 -> int32
matmul bit-exactly at full bf16 throughput. round() is implemented as
(v*scale + 1.5*2^23) - 1.5*2^23 in f32 (round-half-even, matching jnp).

Layout strategy: the TensorEngine contracts along the partition axis, so
both operands need k-major layout. kernel() passes x.T / w.T slices
(numpy prep); the device does only contiguous/strided loads, elementwise
quantize (ScalarE scale+magic, VectorE subtract+bf16-cast), matmuls and
stores. No device transposes, no large collectives: each core re-reads
all of w.T and quantizes it panel-by-panel, pipelined under the matmuls.
amax is computed on-device: per-core abs-max over disjoint slices of x
and w (VectorE for x, GpSimd full-reduce for w), then one AllReduce-max
collective of 2 scalars.

x row-parallel: core c computes out rows [c*1024, (c+1)*1024).
"""

import numpy as np

import concourse.bass as bass
import concourse.bacc as bacc
import concourse.bass_isa as bass_isa
import concourse.mybir as mybir
import concourse.tile as tile
from concourse.bass_utils import run_bass_kernel_spmd

N, K, M = 8192, 4096, 4096
N_CORES = 8
NS = N // N_CORES   # 1024 x rows per core
WS = M // N_CORES   # 512 w rows per core (amax shard)
P = 128
KB = K // P         # 32 k-blocks
NB = NS // P        # 8 n-blocks per core
MP = 512            # m-panel width
NMP = M // MP       # 8 m-panels

MAGIC = 12582912.0  # 1.5 * 2**23
F32 = mybir.dt.float32
BF16 = mybir.dt.bfloat16

_cached_nc = None


def _body(nc, tc, xs, wa, wf, bias_in, out):
    RG = [list(range(N_CORES))]
    # xs: [K, NS] f32 (x.T slice)   -> tiles [128, 4, NS], k on partitions
    # wa: [K, WS] f32 (w.T slice)   -> tiles [128, 8, MP] (amax only)
    # wf: [K, M]  f32 (full w.T)    -> per-panel tiles [128, 8, MP]
    xs_t = xs.rearrange("(t a p) n -> t p a n", a=4, p=P)   # [8, 128, 4, 1024]
    wa_t = wa.rearrange("(h j p) m -> h p j m", j=8, p=P)   # [4, 128, 8, 512]

    with (
        tc.tile_pool(name="const", bufs=1) as const,
        tc.tile_pool(name="dram", bufs=1, space="DRAM") as dram,
        tc.tile_pool(name="ld", bufs=3) as ld,
        tc.tile_pool(name="xt", bufs=1) as xtp,
        tc.tile_pool(name="wt", bufs=6) as wtp,
        tc.tile_pool(name="ps", bufs=4, space="PSUM") as psp,
        tc.tile_pool(name="ob", bufs=3) as obp,
    ):
        cc_in = dram.tile([1, 16], F32)
        cc_out = dram.tile([1, 16], F32, addr_space="Shared")
        warm_in = dram.tile([1, 16], F32)
        warm_out = dram.tile([1, 16], F32, addr_space="Shared")

        # warm up the collective stream (ncfw init) under the amax scan
        warm = const.tile([1, 16], F32)
        nc.vector.memset(warm[:], 0.0)
        nc.gpsimd.dma_start(warm_in[:], warm[:])
        nc.gpsimd.collective_compute(
            "AllReduce", mybir.AluOpType.max,
            ins=[warm_in.opt()], outs=[warm_out.opt()], replica_groups=RG,
        )

        bias_bc = const.tile([P, M], F32)

        scl = const.tile([P, 4], F32)   # 0:scale_x 1:scale_w 2:dequant 3:tmp
        magic = const.tile([P, 1], F32)
        nc.vector.memset(magic[:], MAGIC)

        # ---- Phase A: local abs-max over this core's distinct slices ----
        partx = const.tile([P, 8], F32)
        partw = const.tile([P, 4], F32)
        for h in range(4):
            tl = ld.tile([P, 8, MP], F32, tag="ldf32", name=f"ldw{h}")
            nc.scalar.dma_start(tl[:], wa_t[h])
            nc.vector.tensor_reduce(
                out=partw[:, h : h + 1], in_=tl[:], op=mybir.AluOpType.max,
                axis=mybir.AxisListType.XY, apply_absolute_value=True,
            )
        for t in range(8):
            tl = ld.tile([P, 4, NS], F32, tag="ldf32", name=f"ldx{t}")
            nc.sync.dma_start(tl[:], xs_t[t])
            nc.vector.tensor_reduce(
                out=partx[:, t : t + 1], in_=tl[:], op=mybir.AluOpType.max,
                axis=mybir.AxisListType.XY, apply_absolute_value=True,
            )
        pxw = const.tile([P, 2], F32)
        nc.vector.tensor_reduce(out=pxw[:, 0:1], in_=partx[:], op=mybir.AluOpType.max,
                                axis=mybir.AxisListType.X)
        nc.vector.tensor_reduce(out=pxw[:, 1:2], in_=partw[:], op=mybir.AluOpType.max,
                                axis=mybir.AxisListType.X)
        rxw = const.tile([P, 2], F32)
        nc.gpsimd.partition_all_reduce(rxw[:], pxw[:], channels=P,
                                       reduce_op=bass_isa.ReduceOp.max)

        # ---- AllReduce(max) of [amax_x, amax_w] across the 8 cores ----
        pack = const.tile([1, 16], F32)
        nc.vector.memset(pack[:], 0.0)
        nc.vector.tensor_copy(pack[:1, 0:2], rxw[:1, 0:2])
        nc.gpsimd.dma_start(cc_in[:], pack[:])
        nc.gpsimd.collective_compute(
            "AllReduce", mybir.AluOpType.max,
            ins=[cc_in.opt()], outs=[cc_out.opt()], replica_groups=RG,
        )
        got = const.tile([1, 16], F32)
        nc.gpsimd.dma_start(got[:], cc_out[:])
        gb = const.tile([P, 16], F32)
        nc.gpsimd.partition_broadcast(gb[:], got[:])

        bias_b_ap = bass.AP(
            tensor=bias_in.tensor,
            offset=bias_in.offset,
            ap=[[0, P]] + list(bias_in.ap),
        )
        nc.gpsimd.dma_start(out=bias_bc[:], in_=bias_b_ap)

        inv = const.tile([P, 2], F32)
        nc.vector.reciprocal(inv[:], gb[:, 0:2])
        nc.vector.tensor_scalar(out=scl[:, 0:2], in0=inv[:], scalar1=127.0,
                                scalar2=None, op0=mybir.AluOpType.mult)
        nc.vector.tensor_tensor(out=scl[:, 3:4], in0=gb[:, 0:1], in1=gb[:, 1:2],
                                op=mybir.AluOpType.mult)
        nc.vector.tensor_scalar(out=scl[:, 2:3], in0=scl[:, 3:4],
                                scalar1=float(np.float32(1.0) / np.float32(16129.0)),
                                scalar2=None, op0=mybir.AluOpType.mult)

        xT = xtp.tile([P, KB, NS], BF16)  # resident quantized x.T (8.4 MB)

        def quant_x_tile(t):
            tl = ld.tile([P, 4, NS], F32, tag="ldf32", name=f"ldx2{t}")
            nc.sync.dma_start(tl[:], xs_t[t])
            nc.vector.tensor_scalar(out=tl[:], in0=tl[:], scalar1=scl[:, 0:1],
                                    scalar2=MAGIC, op0=mybir.AluOpType.mult,
                                    op1=mybir.AluOpType.add)
            nc.vector.tensor_scalar(out=xT[:, 4 * t : 4 * t + 4, :], in0=tl[:],
                                    scalar1=MAGIC, scalar2=None,
                                    op0=mybir.AluOpType.subtract)

        quant_x_tile(0)

        def quant_w_panel(p):
            wth = []
            for h in range(4):
                tl = ld.tile([P, 8, MP], F32, tag="ldf32", name=f"ldwp{p}_{h}")
                src = bass.AP(
                    tensor=wf.tensor,
                    offset=wf.offset + h * (K // 4) * M + p * MP,
                    ap=[[M, P], [P * M, 8], [1, MP]],
                )
                nc.scalar.dma_start(tl[:], src)
                nc.vector.tensor_scalar(out=tl[:], in0=tl[:], scalar1=scl[:, 1:2],
                                        scalar2=MAGIC, op0=mybir.AluOpType.mult,
                                        op1=mybir.AluOpType.add)
                w = wtp.tile([P, 8, MP], BF16, tag="wT", name=f"wT{p}_{h}")
                nc.vector.tensor_scalar(out=w[:], in0=tl[:], scalar1=MAGIC,
                                        scalar2=None, op0=mybir.AluOpType.subtract)
                wth.append(w)
            return wth

        panel_w = {0: quant_w_panel(0)}
        for t in range(1, 8):
            quant_x_tile(t)

        # ---- main loop: matmuls + epilogue (w quant pipelined one ahead) ----
        for p in range(NMP):
            wth = panel_w.pop(p)
            if p + 1 < NMP:
                panel_w[p + 1] = quant_w_panel(p + 1)
            for nb in range(NB):
                ps = psp.tile([P, MP], F32, tag="ps", name=f"ps{p}_{nb}")
                for i in range(KB):
                    ks = (4 * nb + i) % KB
                    nc.tensor.matmul(
                        ps[:], xT[:, ks, nb * P : (nb + 1) * P],
                        wth[ks // 8][:, ks % 8, :],
                        start=(i == 0), stop=(i == KB - 1),
                    )
                ob = obp.tile([P, MP], F32, tag="ob", name=f"ob{p}_{nb}")
                nc.vector.scalar_tensor_tensor(
                    out=ob[:], in0=ps[:], scalar=scl[:, 2:3],
                    in1=bias_bc[:, p * MP : (p + 1) * MP],
                    op0=mybir.AluOpType.mult, op1=mybir.AluOpType.add,
                )
                nc.gpsimd.dma_start(
                    out[nb * P : (nb + 1) * P, p * MP : (p + 1) * MP], ob[:]
                )


def _build():
    global _cached_nc
    if _cached_nc is not None:
        return _cached_nc
    nc = bacc.Bacc("TRN2", target_bir_lowering=False, debug=False,
                   num_devices=N_CORES)
    xs = nc.dram_tensor("xs", [K, NS], F32, kind="ExternalInput")
    wa = nc.dram_tensor("wa", [K, WS], F32, kind="ExternalInput")
    wf = nc.dram_tensor("wf", [K, M], F32, kind="ExternalInput")
    bias = nc.dram_tensor("bias", [M], F32, kind="ExternalInput")
    out = nc.dram_tensor("out", [NS, M], F32, kind="ExternalOutput")
    with tile.TileContext(nc) as tc:
        _body(nc, tc, xs.ap(), wa.ap(), wf.ap(), bias.ap(), out.ap())
    nc.compile()
    _cached_nc = nc
    return nc


def kernel(x, weight, bias, _trace=False, _trace_kwargs=None):
    x = np.asarray(x, dtype=np.float32)
    weight = np.asarray(weight, dtype=np.float32)
    bias = np.ascontiguousarray(np.asarray(bias, dtype=np.float32))
    assert x.shape == (N, K) and weight.shape == (M, K) and bias.shape == (M,)

    nc = _build()
    xt = x.T          # [K, N] view
    wt = np.ascontiguousarray(weight.T)  # [K, M]
    in_maps = [
        {
            "xs": np.ascontiguousarray(xt[:, c * NS : (c + 1) * NS]),
            "wa": np.ascontiguousarray(wt[:, c * WS : (c + 1) * WS]),
            "wf": wt,
            "bias": bias,
        }
        for c in range(N_CORES)
    ]
    res = run_bass_kernel_spmd(
        nc, in_maps, core_ids=list(range(N_CORES)),
        trace=_trace, **(_trace_kwargs or {}),
    )
    out = np.concatenate([res.results[c]["out"] for c in range(N_CORES)], axis=0)
    if _trace:
        return out, res
    return out
